# revision 1
# baseline (speedup 1.0000x reference)
"""Trainium2 Bass kernel for DynamicDirectionalConv.

Math (per batch b):
  x_low = einsum('chw,mc->mhw', x, w_reduce)                 # 1x1 reduce C=256->16
  w_h   = cos(angle)^2
  out_low = w_h * (x_low (*) BASE_H) + (1-w_h) * (x_low (*) BASE_V)
  out   = einsum('mhw,cm->chw', out_low, w_expand)           # 1x1 expand 16->256

The per-pixel blend factors out of the tap sum (weights multiply at the
output pixel), and both base kernels are axis-aligned anisotropic
Gaussians -> separable rank-1 7x7 convs with reflect padding.

Sharding: data-parallel over batch, 1 batch per NeuronCore (B=8, 8 cores).

Layout pipeline (per core):
  x [c, h, w] --PE (x tiles as stationary)--> X3 [w, (h_pad, m)]
  H-pass: symmetric-tap FMA chains (gpsimd pair-sums + DVE)
  W-pass: banded reflect matrices via PE (lhsT = Tw.T, stationary)
  blend with cos^2(angle) replicated over m
  pack-transpose on PE -> [(hl8, m), (k, w)], h = 8k + hl
  expand: K=64 bf16 matmuls with zero-padded weight variants
"""

import math

import numpy as np

import concourse.bass as bass
import concourse.tile as tile
from concourse import mybir
import bass_rust
from concourse.bass_utils import run_bass_kernel_spmd

B, C, H, W, MID = 8, 256, 128, 128, 16
K, PAD = 7, 3
F32 = mybir.dt.float32
BF16 = mybir.dt.bfloat16

ALL_STAGES = frozenset(
    ["wh", "indma", "s1", "conv", "wpass", "blend", "pack", "s4", "outdma"]
)


# ----------------------------------------------------------------- host consts
def _host_consts():
    ax = np.linspace(-(K // 2), K // 2, K, dtype=np.float64)
    e_w = np.exp(-(ax**2) / (2 * 2.5**2))  # wide gaussian (sigma_h)
    e_n = np.exp(-(ax**2) / (2 * 1.0**2))  # narrow gaussian (sigma_v)
    # BASE_H[i,j] = e_w[i]*e_n[j]/(S+eps); BASE_V[i,j] = e_n[i]*e_w[j]/(S+eps)
    s_h = float((np.outer(e_w, e_n)).sum()) + 1e-8
    s_v = float((np.outer(e_n, e_w)).sum()) + 1e-8
    gh_A = e_w.astype(np.float32)
    gh_B = e_n.astype(np.float32)
    gw_A = (e_n / s_h).astype(np.float64)
    gw_B = (e_w / s_v).astype(np.float64)

    def refl(t):
        if t < 0:
            return -t
        if t > W - 1:
            return 2 * (W - 1) - t
        return t

    def banded(g):
        T = np.zeros((W, W), dtype=np.float64)
        for wo in range(W):
            for j in range(K):
                T[wo, refl(wo + j - PAD)] += g[j]
        return T.astype(np.float32)

    TwA = banded(gw_A)  # out = TwA @ Y  (w-conv with reflect)
    TwB = banded(gw_B)
    ident = np.eye(128, dtype=np.float32)
    return gh_A, gh_B, np.ascontiguousarray(TwA.T), np.ascontiguousarray(TwB.T), ident


GH_A, GH_B, TWTA, TWTB, IDENT = _host_consts()


# ----------------------------------------------------------------- bass module
def build_nc(split_multiwaits=True, reps=1, loop_n=1, stages=ALL_STAGES,
             use_gpsimd=True):
    st = frozenset(stages)
    nc = bass.Bass()

    x_in = nc.dram_tensor("x", [C, H, W], F32, kind="ExternalInput")
    ang_in = nc.dram_tensor("angle", [H, W], F32, kind="ExternalInput")
    wrT0_in = nc.dram_tensor("wrT0", [128, MID], F32, kind="ExternalInput")
    wrT1_in = nc.dram_tensor("wrT1", [128, MID], F32, kind="ExternalInput")
    twtA_in = nc.dram_tensor("TwTA", [128, 128], F32, kind="ExternalInput")
    twtB_in = nc.dram_tensor("TwTB", [128, 128], F32, kind="ExternalInput")
    ident_in = nc.dram_tensor("ident", [128, 128], F32, kind="ExternalInput")
    wet_in = nc.dram_tensor("WETrep", [128, 4 * C], BF16, kind="ExternalInput")
    out_dram = nc.dram_tensor("out", [C, H, W], F32, kind="ExternalOutput")

    HP = H + 2 * PAD  # 134 padded rows

    from contextlib import ExitStack

    with tile.TileContext(nc) as tc, ExitStack() as es:
        consts = es.enter_context(tc.tile_pool(name="consts", bufs=1))
        xpool = es.enter_context(tc.tile_pool(name="xpool", bufs=4))
        x3pool = es.enter_context(tc.tile_pool(name="x3", bufs=1))
        ypool = es.enter_context(tc.tile_pool(name="y", bufs=4))
        zpool = es.enter_context(tc.tile_pool(name="z", bufs=4))
        bpool = es.enter_context(tc.tile_pool(name="blend", bufs=4))
        olppool = es.enter_context(tc.tile_pool(name="olp", bufs=1))
        opool = es.enter_context(tc.tile_pool(name="ostage", bufs=6))
        whpool = es.enter_context(tc.tile_pool(name="wh", bufs=2))
        ps1pool = es.enter_context(tc.tile_pool(name="ps1", bufs=2, space="PSUM"))
        pswpool = es.enter_context(tc.tile_pool(name="psw", bufs=2, space="PSUM"))
        pstpool = es.enter_context(tc.tile_pool(name="pst", bufs=1, space="PSUM"))
        psopool = es.enter_context(tc.tile_pool(name="pso", bufs=3, space="PSUM"))

        # ---- constants to SBUF (once)
        wrT0 = consts.tile([128, MID], F32)
        wrT1 = consts.tile([128, MID], F32)
        twtA = consts.tile([128, 128], F32)
        twtB = consts.tile([128, 128], F32)
        ident = consts.tile([128, 128], F32)
        wet = consts.tile([128, 4 * C], BF16)
        nc.sync.dma_start(out=wrT0, in_=wrT0_in[:])
        nc.sync.dma_start(out=wrT1, in_=wrT1_in[:])
        nc.sync.dma_start(out=twtA, in_=twtA_in[:])
        nc.sync.dma_start(out=twtB, in_=twtB_in[:])
        nc.sync.dma_start(out=ident, in_=ident_in[:])
        nc.sync.dma_start(out=wet, in_=wet_in[:])

        loop_cm = tc.For_i(0, loop_n, 1) if loop_n > 1 else None
        if loop_cm is not None:
            es.enter_context(loop_cm)

        for _rep in range(reps):
            # ---- w_h = cos(angle)^2 -> transposed + replicated over m.
            # host passes angle pre-mapped to wrap(2a + pi/2);
            # cos(a)^2 = 0.5 + 0.5*sin(2a + pi/2)
            whrep = consts.tile([128, H * MID], F32)  # [w, (h, m)]
            if "wh" in st:
                ang = whpool.tile([128, W], F32)  # [h, w]
                nc.sync.dma_start(out=ang, in_=ang_in[:])
                csq = whpool.tile([128, W], F32)
                nc.scalar.activation(
                    csq, ang, mybir.ActivationFunctionType.Sin,
                    bias=0.0, scale=1.0,
                )
                wh_hw = whpool.tile([128, W], F32)
                nc.scalar.activation(
                    wh_hw, csq, mybir.ActivationFunctionType.Copy,
                    bias=0.5, scale=0.5,
                )
                ps_wh = pstpool.tile([128, 512], F32, tag="pst")
                nc.tensor.transpose(ps_wh[:, 0:128], wh_hw, ident)
                whT = whpool.tile([128, 128], F32)  # [w, h]
                nc.scalar.copy(out=whT, in_=ps_wh[:, 0:128])
                whrep_r = whrep.rearrange("p (h m) -> p h m", m=MID)
                for mi in range(MID):
                    nc.vector.tensor_copy(out=whrep_r[:, :, mi], in_=whT)

            X3 = x3pool.tile([128, HP * MID], F32)  # [w, (hp, m)]
            X3r = X3.rearrange("p (hp m) -> p hp m", m=MID)
            OLp = olppool.tile([128, 16 * W], BF16)

            def emit_s1_group(hg):
                """x_low for 32 h rows -> X3 interior rows."""
                h0 = hg * 32
                ps1 = ps1pool.tile([128, 512], F32, tag="ps1")
                for sub in range(2):  # two 16-row DMA tiles per group
                    hh = h0 + sub * 16
                    xt0 = xpool.tile([128, 16, W], F32, tag="xt0")
                    xt1 = xpool.tile([128, 16, W], F32, tag="xt1")
                    if "indma" in st:
                        nc.sync.dma_start(out=xt0, in_=x_in[0:128, hh:hh + 16, :])
                        nc.sync.dma_start(out=xt1, in_=x_in[128:256, hh:hh + 16, :])
                    if "s1" in st:
                        for hl in range(16):
                            fo = (sub * 16 + hl) * MID
                            nc.tensor.matmul(
                                ps1[:, fo:fo + MID], lhsT=xt0[:, hl, :],
                                rhs=wrT0, start=True, stop=False,
                            )
                            nc.tensor.matmul(
                                ps1[:, fo:fo + MID], lhsT=xt1[:, hl, :],
                                rhs=wrT1, start=False, stop=True,
                            )
                if "s1" in st:
                    nc.scalar.copy(
                        out=X3[:, (PAD + h0) * MID:(PAD + h0 + 32) * MID],
                        in_=ps1,
                    )
                    if hg == 0:
                        # top reflect: hp 0,1,2 <- hp 6,5,4  (h -k <- h k)
                        for k in range(1, PAD + 1):
                            nc.scalar.copy(
                                out=X3r[:, PAD - k, :], in_=X3r[:, PAD + k, :]
                            )
                    if hg == 3:
                        # bottom reflect: h 127+k <- h 127-k
                        for k in range(1, PAD + 1):
                            nc.scalar.copy(
                                out=X3r[:, PAD + H - 1 + k, :],
                                in_=X3r[:, PAD + H - 1 - k, :],
                            )

            def emit_chunk(ch):
                """conv + blend + pack + expand + store for 32 output rows."""
                h0 = ch * 32

                def xsl(i):
                    return X3[:, (h0 + i) * MID:(h0 + i) * MID + 512]

                Ya = ypool.tile([128, 512], F32, tag="ya")
                Yb = ypool.tile([128, 512], F32, tag="yb")
                if "conv" in st:
                    # symmetric taps: shared pair sums on gpsimd, FMA on DVE
                    add_eng = nc.gpsimd if use_gpsimd else nc.vector
                    s_tiles = []
                    for i in range(3):
                        s = ypool.tile([128, 512], F32, tag=f"s{i}")
                        add_eng.tensor_add(
                            out=s, in0=xsl(i), in1=xsl(K - 1 - i)
                        )
                        s_tiles.append(s)
                    nc.vector.tensor_scalar_mul(Ya, xsl(3), float(GH_A[3]))
                    nc.vector.tensor_scalar_mul(Yb, xsl(3), float(GH_B[3]))
                    for i in range(3):
                        nc.vector.scalar_tensor_tensor(
                            out=Ya, in0=s_tiles[i], scalar=float(GH_A[i]),
                            in1=Ya, op0=mybir.AluOpType.mult,
                            op1=mybir.AluOpType.add,
                        )
                        nc.vector.scalar_tensor_tensor(
                            out=Yb, in0=s_tiles[i], scalar=float(GH_B[i]),
                            in1=Yb, op0=mybir.AluOpType.mult,
                            op1=mybir.AluOpType.add,
                        )
                # W-pass: Za = TwA @ Ya, Zb = TwB @ Yb
                Za = zpool.tile([128, 512], F32, tag="za")
                Zb = zpool.tile([128, 512], F32, tag="zb")
                if "wpass" in st:
                    psa = pswpool.tile([128, 512], F32, tag="psw")
                    nc.tensor.matmul(psa, lhsT=twtA, rhs=Ya, start=True, stop=True)
                    nc.scalar.copy(out=Za, in_=psa)
                    psb = pswpool.tile([128, 512], F32, tag="psw")
                    nc.tensor.matmul(psb, lhsT=twtB, rhs=Yb, start=True, stop=True)
                    nc.vector.tensor_copy(out=Zb, in_=psb)
                # blend: OL = Zb + whrep*(Za - Zb)
                OL = bpool.tile([128, 512], F32, tag="ol")
                if "blend" in st:
                    d = bpool.tile([128, 512], F32, tag="d")
                    nc.vector.tensor_sub(out=d, in0=Za, in1=Zb)
                    p = bpool.tile([128, 512], F32, tag="p")
                    nc.vector.tensor_mul(
                        out=p, in0=d, in1=whrep[:, h0 * MID:h0 * MID + 512]
                    )
                    nc.vector.tensor_add(out=OL, in0=p, in1=Zb)
                # pack-transpose: [w, (8h,16m)] blocks -> [(8h,16m), w]
                if "pack" in st:
                    pst = pstpool.tile([128, 512], F32, tag="pst")
                    for kb in range(4):
                        nc.tensor.transpose(
                            pst[:, kb * 128:(kb + 1) * 128],
                            OL[:, kb * 128:(kb + 1) * 128],
                            ident,
                        )
                    nc.scalar.copy(
                        out=OLp[:, ch * 512:(ch + 1) * 512], in_=pst
                    )  # f32 -> bf16
                # expand: out[c, (k, w)] = w_expand @ out_low; K=64 matmuls
                # with zeros outside the hl-selected 16-row block
                for hl in range(8):
                    b = 64 * (hl // 4)
                    v = hl % 4
                    for cc in range(2):
                        ost = opool.tile([128, 512], F32, tag="ost")
                        if "s4" in st:
                            pso = psopool.tile([128, 512], F32, tag="pso")
                            nc.tensor.matmul(
                                pso,
                                lhsT=wet[b:b + 64,
                                         (v * 2 + cc) * 128:(v * 2 + cc + 1) * 128],
                                rhs=OLp[b:b + 64, ch * 512:(ch + 1) * 512],
                                start=True, stop=True,
                                tile_position=(b, 0),
                            )
                            if (hl + cc) % 2 == 0:
                                nc.scalar.copy(out=ost, in_=pso)
                            else:
                                nc.vector.tensor_copy(out=ost, in_=pso)
                        if "outdma" in st:
                            # rows h = 8k + hl, k in [4ch, 4ch+4)
                            o_r = out_dram[cc * 128:(cc + 1) * 128].rearrange(
                                "c (k j) w -> c k j w", j=8
                            )
                            src = ost if "s4" in st else whrep[:, 0:512]
                            nc.sync.dma_start(
                                out=o_r[:, 4 * ch:4 * ch + 4, hl, :],
                                in_=src.rearrange("c (k w) -> c k w", w=W),
                            )

            # interleaved emission: chunk ch depends on s1 groups ch and ch+1
            emit_s1_group(0)
            emit_s1_group(1)
            emit_chunk(0)
            emit_s1_group(2)
            emit_chunk(1)
            emit_s1_group(3)
            emit_chunk(2)
            emit_chunk(3)

    if split_multiwaits:
        _split_multiwaits(nc)
    return nc


def _split_multiwaits(nc):
    """Walrus in this toolchain accepts at most one sync-wait per
    instruction; hoist extras onto same-engine nops just before it."""
    n_new = 0
    for f in nc.m.functions:
        for bb in f.blocks:
            out, changed = [], False
            for ins in bb.instructions:
                si = ins.sync_info
                if si is not None and len(si.on_wait) > 1:
                    changed = True
                    waits = list(si.on_wait)
                    for w in waits[:-1]:
                        n_new += 1
                        nop = bass_rust.InstNoOp(
                            name=f"I-mwsplit-{n_new}", engine=ins.engine
                        )
                        nop.sync_info = mybir.SyncInfo(on_wait=[w], on_update=[])
                        out.append(nop)
                    ins.sync_info = mybir.SyncInfo(
                        on_wait=[waits[-1]], on_update=list(si.on_update)
                    )
                out.append(ins)
            if changed:
                bb.instructions = out
    return n_new


_NC = None


def _get_nc():
    global _NC
    if _NC is None:
        _NC = build_nc()
    return _NC


def make_in_maps(x, angle_map, w_reduce, w_expand):
    wrT = np.ascontiguousarray(w_reduce.T.astype(np.float32))  # [C, MID]
    # wet_ext[p, v*C + c] = w_expand[c, p%16] if (p//16)%4 == v else 0
    wet_rep = np.zeros((128, 4 * C), np.float32)
    weT = w_expand.T.astype(np.float32)  # [MID, C]
    for p in range(128):
        v = (p // 16) % 4
        wet_rep[p, v * C:(v + 1) * C] = weT[p % 16]
    wet_rep = np.ascontiguousarray(wet_rep).astype(mybir.dt.np(BF16))
    consts = {
        "wrT0": wrT[0:128],
        "wrT1": wrT[128:256],
        "TwTA": TWTA,
        "TwTB": TWTB,
        "ident": IDENT,
        "WETrep": wet_rep,
    }
    return [
        {
            "x": np.ascontiguousarray(x[i]),
            "angle": np.ascontiguousarray(
                (
                    np.mod(
                        2.0 * angle_map[i].astype(np.float64)
                        + math.pi / 2 + math.pi,
                        2 * math.pi,
                    )
                    - math.pi
                ).astype(np.float32)
            ),
            **consts,
        }
        for i in range(B)
    ]


def kernel(x, angle_map, w_reduce, w_expand):
    nc = _get_nc()
    in_maps = make_in_maps(x, angle_map, w_reduce, w_expand)
    res = run_bass_kernel_spmd(nc, in_maps, core_ids=list(range(B)))
    return np.stack([r["out"] for r in res.results]).astype(np.float32)



# revision 9
# speedup vs baseline: 1.9731x; 1.9731x over previous
"""Trainium2 Bass kernel for DynamicDirectionalConv.

Math (per batch b):
  x_low = einsum('chw,mc->mhw', x, w_reduce)                 # 1x1 reduce C=256->16
  w_h   = cos(angle)^2
  out_low = w_h * (x_low (*) BASE_H) + (1-w_h) * (x_low (*) BASE_V)
  out   = einsum('mhw,cm->chw', out_low, w_expand)           # 1x1 expand 16->256

The per-pixel blend factors out of the tap sum, and both base kernels
are axis-aligned separable Gaussians -> rank-1 7x7 convs, reflect pad.

Sharding: data-parallel over batch, 1 batch per NeuronCore (B=8).

Layout pipeline (per core, per 32-row h-chunk):
  x [c, (h,w)] bf16 --PE stream (wrT stationary)--> x_low [m, (h,w)] bf16
  --XBAR dma transpose--> X3 [w, (hp, m)] bf16 (reflect-padded rows)
  H-pass: symmetric-tap FMA chains (gpsimd pair-sums + DVE STT), bf16
  W-pass: banded reflect matrices on PE, rhs streamed in h-shuffled
    column order (hl*128 + hh*16 + m, h_local = 4*hh + hl)
  blend with cos^2(angle) (whrep pre-shuffled to match)
  --XBAR dma transpose--> OLp [(hh,m), (hl, w)] bf16
  expand: 16 matmuls/chunk, zero-padded weight variants select hh;
    out rows h-contiguous in groups of 4 -> big contiguous out DMA
"""

import math

import numpy as np

import concourse.bass as bass
import concourse.tile as tile
from concourse import mybir
import bass_rust
from concourse.bass_utils import run_bass_kernel_spmd

B, C, H, W, MID = 8, 256, 128, 128, 16
K, PAD = 7, 3
F32 = mybir.dt.float32
F32R = mybir.dt.float32r
BF16 = mybir.dt.bfloat16
NPBF = mybir.dt.np(BF16)

HP = H + 2 * PAD  # 134 padded rows
CH = 32           # h rows per chunk
NCH = H // CH     # 4 chunks

ALL_STAGES = frozenset(
    ["wh", "indma", "reduce", "xbar1", "hconv", "wpass", "blend",
     "xbar2", "expand", "outdma"]
)


# ----------------------------------------------------------------- host consts
def _host_consts():
    ax = np.linspace(-(K // 2), K // 2, K, dtype=np.float64)
    e_w = np.exp(-(ax**2) / (2 * 2.5**2))  # wide gaussian (sigma 2.5)
    e_n = np.exp(-(ax**2) / (2 * 1.0**2))  # narrow gaussian (sigma 1.0)
    # BASE_H[i,j] = e_w[i]*e_n[j]/(S+eps); BASE_V[i,j] = e_n[i]*e_w[j]/(S+eps)
    s_h = float((np.outer(e_w, e_n)).sum()) + 1e-8
    s_v = float((np.outer(e_n, e_w)).sum()) + 1e-8
    gh_A = e_w.astype(np.float32)          # h-taps, kernel A
    gh_B = e_n.astype(np.float32)          # h-taps, kernel B
    gw_A = e_n / s_h                       # w-taps (normalized), kernel A
    gw_B = e_w / s_v

    def refl(t):
        if t < 0:
            return -t
        if t > W - 1:
            return 2 * (W - 1) - t
        return t

    def banded(g):
        T = np.zeros((W, W), dtype=np.float64)
        for wo in range(W):
            for j in range(K):
                T[wo, refl(wo + j - PAD)] += g[j]
        return T.astype(np.float32)

    TwA = banded(gw_A)  # out = TwA @ Y  (w-conv with reflect)
    TwB = banded(gw_B)
    ident = np.eye(128, dtype=np.float32)
    return gh_A, gh_B, np.ascontiguousarray(TwA.T), np.ascontiguousarray(TwB.T), ident


GH_A, GH_B, TWTA, TWTB, IDENT = _host_consts()


# ----------------------------------------------------------------- bass module
def build_nc(split_multiwaits=True, loop_n=1, stages=ALL_STAGES):
    st = frozenset(stages)
    nc = bass.Bass()

    x_in = nc.dram_tensor("x", [C, H, W], BF16, kind="ExternalInput")
    ang_in = nc.dram_tensor("angle", [H, W], F32, kind="ExternalInput")
    wrT0_in = nc.dram_tensor("wrT0", [128, MID], BF16, kind="ExternalInput")
    wrT1_in = nc.dram_tensor("wrT1", [128, MID], BF16, kind="ExternalInput")
    twtA_in = nc.dram_tensor("TwTA", [128, 128], BF16, kind="ExternalInput")
    twtB_in = nc.dram_tensor("TwTB", [128, 128], BF16, kind="ExternalInput")
    ident_in = nc.dram_tensor("ident", [128, 128], F32, kind="ExternalInput")
    wet_in = nc.dram_tensor("WET", [128, 16 * 128], BF16, kind="ExternalInput")
    out_dram = nc.dram_tensor("out", [C, H, W], BF16, kind="ExternalOutput")

    from contextlib import ExitStack

    with tile.TileContext(nc) as tc, ExitStack() as es:
        consts = es.enter_context(tc.tile_pool(name="consts", bufs=1))
        xpool = es.enter_context(tc.tile_pool(name="xpool", bufs=3))
        xlpool = es.enter_context(tc.tile_pool(name="xlow", bufs=2))
        x3pool = es.enter_context(tc.tile_pool(name="x3", bufs=1))
        ypool = es.enter_context(tc.tile_pool(name="y", bufs=2))
        bpool = es.enter_context(tc.tile_pool(name="blend", bufs=2))
        olppool = es.enter_context(tc.tile_pool(name="olp", bufs=2))
        opool = es.enter_context(tc.tile_pool(name="ostage", bufs=2))
        whpool = es.enter_context(tc.tile_pool(name="wh", bufs=2))
        psrpool = es.enter_context(tc.tile_pool(name="psr", bufs=2, space="PSUM"))
        pswpool = es.enter_context(tc.tile_pool(name="psw", bufs=2, space="PSUM"))
        psopool = es.enter_context(tc.tile_pool(name="pso", bufs=3, space="PSUM"))

        # ---- constants to SBUF (once)
        wrT0 = consts.tile([128, MID], BF16)
        wrT1 = consts.tile([128, MID], BF16)
        twtA = consts.tile([128, 128], BF16)
        twtB = consts.tile([128, 128], BF16)
        ident = consts.tile([128, 128], F32)
        wet = consts.tile([128, 16 * 128], BF16)
        nc.sync.dma_start(out=wrT0, in_=wrT0_in[:])
        nc.sync.dma_start(out=wrT1, in_=wrT1_in[:])
        nc.sync.dma_start(out=twtA, in_=twtA_in[:])
        nc.sync.dma_start(out=twtB, in_=twtB_in[:])
        nc.sync.dma_start(out=ident, in_=ident_in[:])
        nc.sync.dma_start(out=wet, in_=wet_in[:])

        loop_cm = tc.For_i(0, loop_n, 1) if loop_n > 1 else None
        if loop_cm is not None:
            es.enter_context(loop_cm)

        # ---- w_h = cos(angle)^2, replicated over m in shuffled col order.
        # host passes angle pre-mapped to wrap(2a + pi/2);
        # cos(a)^2 = 0.5 + 0.5*sin(2a + pi/2)
        # whrep[w, ch*512 + hl*128 + hh*16 + m] = wh[h= 32ch+4hh+hl, w]
        # whcrep = same for (1 - w_h); blend reads each PSUM bank alone.
        whrep = consts.tile([128, H * MID], F32)
        whcrep = consts.tile([128, H * MID], F32)
        if "wh" in st:
            ang = whpool.tile([128, W], F32)  # [h, w]
            nc.sync.dma_start(out=ang, in_=ang_in[:])
            csq = whpool.tile([128, W], F32)
            nc.scalar.activation(
                csq, ang, mybir.ActivationFunctionType.Sin,
                bias=0.0, scale=1.0,
            )
            wh_hw = whpool.tile([128, W], F32)
            nc.scalar.activation(
                wh_hw, csq, mybir.ActivationFunctionType.Copy,
                bias=0.5, scale=0.5,
            )
            whc_hw = whpool.tile([128, W], F32)
            nc.scalar.activation(
                whc_hw, csq, mybir.ActivationFunctionType.Copy,
                bias=0.5, scale=-0.5,
            )
            ps_wh = psopool.tile([128, 512], F32, tag="pso")
            nc.tensor.transpose(ps_wh[:, 0:128], wh_hw, ident)
            nc.tensor.transpose(ps_wh[:, 128:256], whc_hw, ident)
            whT = whpool.tile([128, 128], F32)  # [w, h]
            whcT = whpool.tile([128, 128], F32)
            nc.scalar.copy(out=whT, in_=ps_wh[:, 0:128])
            nc.scalar.copy(out=whcT, in_=ps_wh[:, 128:256])
            whTr = whT.rearrange("p (ch hh hl) -> p ch hh hl", ch=NCH, hh=8, hl=4)
            whcTr = whcT.rearrange("p (ch hh hl) -> p ch hh hl", ch=NCH, hh=8, hl=4)
            whr = whrep.rearrange(
                "p (ch hl hh m) -> p ch hh hl m", ch=NCH, hl=4, hh=8, m=MID
            )
            whcr = whcrep.rearrange(
                "p (ch hl hh m) -> p ch hh hl m", ch=NCH, hl=4, hh=8, m=MID
            )
            for mi in range(MID):
                nc.vector.tensor_copy(out=whr[:, :, :, :, mi], in_=whTr)
                nc.gpsimd.tensor_copy(out=whcr[:, :, :, :, mi], in_=whcTr)

        X3 = x3pool.tile([128, HP * MID], BF16)  # [w, (hp, m)]
        X3r = X3.rearrange("p (hp m) -> p hp m", m=MID)

        def emit_reduce(ch):
            """x_low for 32 h rows -> X3 rows (via PE stream + XBAR)."""
            h0 = ch * CH
            xt0 = xpool.tile([128, CH, W], BF16, tag="xt0")
            xt1 = xpool.tile([128, CH, W], BF16, tag="xt1")
            if "indma" in st:
                nc.sync.dma_start(out=xt0, in_=x_in[0:128, h0:h0 + CH, :])
                nc.sync.dma_start(out=xt1, in_=x_in[128:256, h0:h0 + CH, :])
            xl = xlpool.tile([16, CH * W], BF16, tag="xl")
            if "reduce" in st:
                for t in range(8):
                    ps = psrpool.tile([16, 512], F32, tag="psr")
                    nc.tensor.matmul(
                        ps, lhsT=wrT0, rhs=xt0[:, 4 * t:4 * t + 4, :],
                        start=True, stop=False,
                    )
                    nc.tensor.matmul(
                        ps, lhsT=wrT1, rhs=xt1[:, 4 * t:4 * t + 4, :],
                        start=False, stop=True,
                    )
                    nc.scalar.copy(out=xl[:, 512 * t:512 * t + 512], in_=ps)
            if "xbar1" in st:
                xslab = X3[:, (PAD + h0) * MID:(PAD + h0 + CH) * MID].rearrange(
                    "p (h m) -> p h m", m=MID
                )
                nc.sync.dma_start_transpose(out=xslab, in_=xl)
                if ch == 0:
                    # top reflect: hp 0,1,2 <- hp 6,5,4  (h -k <- h k)
                    for k in range(1, PAD + 1):
                        nc.scalar.copy(
                            out=X3r[:, PAD - k, :], in_=X3r[:, PAD + k, :]
                        )
                if ch == NCH - 1:
                    # bottom reflect: h 127+k <- h 127-k
                    for k in range(1, PAD + 1):
                        nc.scalar.copy(
                            out=X3r[:, PAD + H - 1 + k, :],
                            in_=X3r[:, PAD + H - 1 - k, :],
                        )

        def emit_chunk(ch):
            """conv + blend + pack + expand + store for 32 output rows."""
            h0 = ch * CH

            def xsl(i):
                return X3[:, (h0 + i) * MID:(h0 + i) * MID + 512]

            Ya = ypool.tile([128, 512], BF16, tag="ya")
            Yb = ypool.tile([128, 512], BF16, tag="yb")
            if "hconv" in st:
                # symmetric taps: shared pair sums on gpsimd, FMA on DVE
                s_tiles = []
                for i in range(3):
                    s = ypool.tile([128, 512], BF16, tag=f"s{i}")
                    nc.gpsimd.tensor_add(out=s, in0=xsl(i), in1=xsl(K - 1 - i))
                    s_tiles.append(s)
                nc.vector.tensor_scalar_mul(Ya, xsl(3), float(GH_A[3]))
                nc.vector.tensor_scalar_mul(Yb, xsl(3), float(GH_B[3]))
                for i in range(3):
                    nc.vector.scalar_tensor_tensor(
                        out=Ya, in0=s_tiles[i], scalar=float(GH_A[i]),
                        in1=Ya, op0=mybir.AluOpType.mult,
                        op1=mybir.AluOpType.add,
                    )
                    nc.vector.scalar_tensor_tensor(
                        out=Yb, in0=s_tiles[i], scalar=float(GH_B[i]),
                        in1=Yb, op0=mybir.AluOpType.mult,
                        op1=mybir.AluOpType.add,
                    )
            # W-pass: Za = TwA @ Ya, Zb = TwB @ Yb; rhs streamed in
            # h-shuffled order so psum cols are (hl, hh, m)
            psa = pswpool.tile([128, 512], F32, tag="psw")
            psb = pswpool.tile([128, 512], F32, tag="psw")
            if "wpass" in st:
                ya_shuf = Ya.rearrange("p (hh hl m) -> p hl hh m", hh=8, hl=4, m=MID)
                yb_shuf = Yb.rearrange("p (hh hl m) -> p hl hh m", hh=8, hl=4, m=MID)
                nc.tensor.matmul(psa, lhsT=twtA, rhs=ya_shuf, start=True, stop=True)
                nc.tensor.matmul(psb, lhsT=twtB, rhs=yb_shuf, start=True, stop=True)
            # blend: OL = whrep*Za + whcrep*Zb (each op reads one PSUM bank)
            OL = bpool.tile([128, 512], BF16, tag="ol")
            if "blend" in st:
                t1 = bpool.tile([128, 512], F32, tag="t1")
                nc.vector.tensor_mul(
                    out=t1, in0=psa, in1=whrep[:, h0 * MID:h0 * MID + 512]
                )
                t2 = bpool.tile([128, 512], F32, tag="t2")
                nc.vector.tensor_mul(
                    out=t2, in0=psb, in1=whcrep[:, h0 * MID:h0 * MID + 512]
                )
                nc.vector.tensor_add(out=OL, in0=t1, in1=t2)
            # pack: XBAR transpose -> OLp [(hh,m), (hl, w)]
            olp = olppool.tile([128, 4, 128], BF16, tag="olp")
            if "xbar2" in st:
                nc.sync.dma_start_transpose(out=olp, in_=OL)
            # expand: 16 matmuls; WET variant (hh, cc) selects 16 of 128
            # contraction rows; out rows h0+4hh .. h0+4hh+3 contiguous
            ost0 = opool.tile([128, CH * W], BF16, tag="ost0")
            ost1 = opool.tile([128, CH * W], BF16, tag="ost1")
            ost = [ost0, ost1]
            for hh in range(8):
                for cc in range(2):
                    if "expand" in st:
                        pso = psopool.tile([128, 512], F32, tag="pso")
                        nc.tensor.matmul(
                            pso,
                            lhsT=wet[:, (hh * 2 + cc) * 128:(hh * 2 + cc + 1) * 128],
                            rhs=olp,
                            start=True, stop=True,
                        )
                        idx = hh * 2 + cc
                        dst = ost[cc][:, hh * 512:hh * 512 + 512]
                        if idx % 2 == 0:
                            nc.scalar.copy(out=dst, in_=pso)
                        else:
                            nc.vector.tensor_copy(out=dst, in_=pso)
            if "outdma" in st:
                for cc in range(2):
                    nc.sync.dma_start(
                        out=out_dram[cc * 128:(cc + 1) * 128, h0:h0 + CH, :],
                        in_=ost[cc].rearrange("c (h w) -> c h w", w=W),
                    )

        # interleaved emission: chunk ch depends on reduce groups ch and ch+1
        emit_reduce(0)
        emit_reduce(1)
        emit_chunk(0)
        emit_reduce(2)
        emit_chunk(1)
        emit_reduce(3)
        emit_chunk(2)
        emit_chunk(3)

    if split_multiwaits:
        _split_multiwaits(nc)
    return nc


def _split_multiwaits(nc):
    """Walrus in this toolchain accepts at most one sync-wait per
    instruction; hoist extras onto same-engine nops just before it."""
    n_new = 0
    for f in nc.m.functions:
        for bb in f.blocks:
            out, changed = [], False
            for ins in bb.instructions:
                si = ins.sync_info
                if si is not None and len(si.on_wait) > 1:
                    changed = True
                    waits = list(si.on_wait)
                    for w in waits[:-1]:
                        n_new += 1
                        nop = bass_rust.InstNoOp(
                            name=f"I-mwsplit-{n_new}", engine=ins.engine
                        )
                        nop.sync_info = mybir.SyncInfo(on_wait=[w], on_update=[])
                        out.append(nop)
                    ins.sync_info = mybir.SyncInfo(
                        on_wait=[waits[-1]], on_update=list(si.on_update)
                    )
                out.append(ins)
            if changed:
                bb.instructions = out
    return n_new


_NC = None


def _get_nc():
    global _NC
    if _NC is None:
        _NC = build_nc()
    return _NC


def make_in_maps(x, angle_map, w_reduce, w_expand):
    wrT = np.ascontiguousarray(w_reduce.T.astype(np.float32))  # [C, MID]
    # WET[p=(hh,m), (sel*2+cc)*128 + cl] = w_expand[cc*128+cl, m] if hh==sel
    weT = w_expand.T.astype(np.float32)  # [MID, C]
    wet = np.zeros((128, 16 * 128), np.float32)
    for p in range(128):
        hh, m = p // 16, p % 16
        for cc in range(2):
            wet[p, (hh * 2 + cc) * 128:(hh * 2 + cc + 1) * 128] = weT[
                m, cc * 128:(cc + 1) * 128
            ]
    consts = {
        "wrT0": wrT[0:128].astype(NPBF),
        "wrT1": wrT[128:256].astype(NPBF),
        "TwTA": TWTA.astype(NPBF),
        "TwTB": TWTB.astype(NPBF),
        "ident": IDENT,
        "WET": np.ascontiguousarray(wet).astype(NPBF),
    }
    return [
        {
            "x": np.ascontiguousarray(x[i]).astype(NPBF),
            "angle": np.ascontiguousarray(
                (
                    np.mod(
                        2.0 * angle_map[i].astype(np.float64)
                        + math.pi / 2 + math.pi,
                        2 * math.pi,
                    )
                    - math.pi
                ).astype(np.float32)
            ),
            **consts,
        }
        for i in range(B)
    ]


def kernel(x, angle_map, w_reduce, w_expand):
    nc = _get_nc()
    in_maps = make_in_maps(x, angle_map, w_reduce, w_expand)
    res = run_bass_kernel_spmd(nc, in_maps, core_ids=list(range(B)))
    return np.stack([r["out"] for r in res.results]).astype(np.float32)


# revision 14
# speedup vs baseline: 1.9921x; 1.0096x over previous
"""Trainium2 Bass kernel for DynamicDirectionalConv.

Math (per batch b):
  x_low = einsum('chw,mc->mhw', x, w_reduce)                 # 1x1 reduce C=256->16
  w_h   = cos(angle)^2
  out_low = w_h * (x_low (*) BASE_H) + (1-w_h) * (x_low (*) BASE_V)
  out   = einsum('mhw,cm->chw', out_low, w_expand)           # 1x1 expand 16->256

The per-pixel blend factors out of the tap sum, and both base kernels
are axis-aligned separable Gaussians -> rank-1 7x7 convs, reflect pad.

Sharding: data-parallel over batch, 1 batch per NeuronCore (B=8).

Layout pipeline (per core, per 32-row h-chunk):
  x [c, (h,w)] bf16 --PE stream (wrT stationary)--> x_low [m, (h,w)] bf16
  --XBAR dma transpose--> X3 [w, (hp, m)] bf16 (reflect-padded rows)
  H-pass: symmetric-tap FMA chains (gpsimd pair-sums + DVE STT), bf16
  W-pass: banded reflect matrices on PE, rhs streamed in h-shuffled
    column order (hl*128 + hh*16 + m, h_local = 4*hh + hl)
  blend with cos^2(angle) (whrep pre-shuffled to match)
  --XBAR dma transpose--> OLp [(hh,m), (hl, w)] bf16
  expand: 16 matmuls/chunk, zero-padded weight variants select hh;
    out rows h-contiguous in groups of 4 -> big contiguous out DMA
"""

import math

import numpy as np

import concourse.bass as bass
import concourse.tile as tile
from concourse import mybir
import bass_rust
from concourse.bass_utils import run_bass_kernel_spmd

B, C, H, W, MID = 8, 256, 128, 128, 16
K, PAD = 7, 3
F32 = mybir.dt.float32
F32R = mybir.dt.float32r
BF16 = mybir.dt.bfloat16
NPBF = mybir.dt.np(BF16)

HP = H + 2 * PAD  # 134 padded rows
CH = 32           # h rows per chunk
NCH = H // CH     # 4 chunks

ALL_STAGES = frozenset(
    ["wh", "indma", "reduce", "xbar1", "hconv", "wpass", "blend",
     "xbar2", "expand", "outdma"]
)


# ----------------------------------------------------------------- host consts
def _host_consts():
    ax = np.linspace(-(K // 2), K // 2, K, dtype=np.float64)
    e_w = np.exp(-(ax**2) / (2 * 2.5**2))  # wide gaussian (sigma 2.5)
    e_n = np.exp(-(ax**2) / (2 * 1.0**2))  # narrow gaussian (sigma 1.0)
    # BASE_H[i,j] = e_w[i]*e_n[j]/(S+eps); BASE_V[i,j] = e_n[i]*e_w[j]/(S+eps)
    s_h = float((np.outer(e_w, e_n)).sum()) + 1e-8
    s_v = float((np.outer(e_n, e_w)).sum()) + 1e-8
    gh_A = e_w.astype(np.float32)          # h-taps, kernel A
    gh_B = e_n.astype(np.float32)          # h-taps, kernel B
    gw_A = e_n / s_h                       # w-taps (normalized), kernel A
    gw_B = e_w / s_v

    def refl(t):
        if t < 0:
            return -t
        if t > W - 1:
            return 2 * (W - 1) - t
        return t

    def banded(g):
        T = np.zeros((W, W), dtype=np.float64)
        for wo in range(W):
            for j in range(K):
                T[wo, refl(wo + j - PAD)] += g[j]
        return T.astype(np.float32)

    TwA = banded(gw_A)  # out = TwA @ Y  (w-conv with reflect)
    TwB = banded(gw_B)
    ident = np.eye(128, dtype=np.float32)
    return gh_A, gh_B, np.ascontiguousarray(TwA.T), np.ascontiguousarray(TwB.T), ident


GH_A, GH_B, TWTA, TWTB, IDENT = _host_consts()


# ----------------------------------------------------------------- bass module
def build_nc(split_multiwaits=True, loop_n=1, stages=ALL_STAGES):
    st = frozenset(stages)
    nc = bass.Bass()

    x_in = nc.dram_tensor("x", [C, H, W], BF16, kind="ExternalInput")
    ang_in = nc.dram_tensor("angle", [H, W], F32, kind="ExternalInput")
    wrp_in = nc.dram_tensor("WRP", [128, 16 * 128], BF16, kind="ExternalInput")
    twtA_in = nc.dram_tensor("TwTA", [128, 128], BF16, kind="ExternalInput")
    twtB_in = nc.dram_tensor("TwTB", [128, 128], BF16, kind="ExternalInput")
    ident_in = nc.dram_tensor("ident", [128, 128], F32, kind="ExternalInput")
    wet_in = nc.dram_tensor("WET", [128, 16 * 128], BF16, kind="ExternalInput")
    out_dram = nc.dram_tensor("out", [C, H, W], BF16, kind="ExternalOutput")

    from contextlib import ExitStack

    with tile.TileContext(nc) as tc, ExitStack() as es:
        consts = es.enter_context(tc.tile_pool(name="consts", bufs=1))
        xpool = es.enter_context(tc.tile_pool(name="xpool", bufs=3))
        xlpool = es.enter_context(tc.tile_pool(name="xlow", bufs=2))
        x3pool = es.enter_context(tc.tile_pool(name="x3", bufs=1))
        ypool = es.enter_context(tc.tile_pool(name="y", bufs=2))
        bpool = es.enter_context(tc.tile_pool(name="blend", bufs=2))
        olppool = es.enter_context(tc.tile_pool(name="olp", bufs=2))
        opool = es.enter_context(tc.tile_pool(name="ostage", bufs=2))
        whpool = es.enter_context(tc.tile_pool(name="wh", bufs=2))
        psrpool = es.enter_context(tc.tile_pool(name="psr", bufs=2, space="PSUM"))
        pswpool = es.enter_context(tc.tile_pool(name="psw", bufs=2, space="PSUM"))
        psopool = es.enter_context(tc.tile_pool(name="pso", bufs=3, space="PSUM"))

        # ---- x tiles for the first two chunks first (head latency), then
        # constants; HW spreads same-engine DMAs across its 16 queues.
        xts = {}

        def emit_xdma(ch):
            h0 = ch * CH
            xt0 = xpool.tile([128, CH, W], BF16, tag="xt0")
            xt1 = xpool.tile([128, CH, W], BF16, tag="xt1")
            if "indma" in st:
                for r in range(2):
                    hr = h0 + r * 16
                    nc.sync.dma_start(out=xt0[:, r * 16:r * 16 + 16, :],
                                      in_=x_in[0:128, hr:hr + 16, :])
                    nc.sync.dma_start(out=xt1[:, r * 16:r * 16 + 16, :],
                                      in_=x_in[128:256, hr:hr + 16, :])
            xts[ch] = (xt0, xt1)

        emit_xdma(0)
        emit_xdma(1)

        wrp = consts.tile([128, 16 * 128], BF16)
        twtA = consts.tile([128, 128], BF16)
        twtB = consts.tile([128, 128], BF16)
        ident = consts.tile([128, 128], F32)
        wet = consts.tile([128, 16 * 128], BF16)
        nc.sync.dma_start(out=wrp, in_=wrp_in[:])
        nc.sync.dma_start(out=twtA, in_=twtA_in[:])
        nc.sync.dma_start(out=twtB, in_=twtB_in[:])
        nc.sync.dma_start(out=ident, in_=ident_in[:])
        nc.sync.dma_start(out=wet, in_=wet_in[:])

        loop_cm = tc.For_i(0, loop_n, 1) if loop_n > 1 else None
        if loop_cm is not None:
            es.enter_context(loop_cm)

        # ---- w_h = cos(angle)^2, replicated over m in shuffled col order.
        # host passes angle pre-mapped to wrap(2a + pi/2);
        # cos(a)^2 = 0.5 + 0.5*sin(2a + pi/2)
        # whrep[w, ch*512 + hl*128 + hh*16 + m] = wh[h= 32ch+4hh+hl, w]
        # whcrep = same for (1 - w_h); blend reads each PSUM bank alone.
        whrep = consts.tile([128, H * MID], F32)
        whcrep = consts.tile([128, H * MID], F32)
        if "wh" in st:
            ang = whpool.tile([128, W], F32)  # [h, w]
            nc.sync.dma_start(out=ang, in_=ang_in[:])
            csq = whpool.tile([128, W], F32)
            nc.scalar.activation(
                csq, ang, mybir.ActivationFunctionType.Sin,
                bias=0.0, scale=1.0,
            )
            wh_hw = whpool.tile([128, W], F32)
            nc.scalar.activation(
                wh_hw, csq, mybir.ActivationFunctionType.Copy,
                bias=0.5, scale=0.5,
            )
            whc_hw = whpool.tile([128, W], F32)
            nc.scalar.activation(
                whc_hw, csq, mybir.ActivationFunctionType.Copy,
                bias=0.5, scale=-0.5,
            )
            ps_wh = psopool.tile([128, 512], F32, tag="pso")
            nc.tensor.transpose(ps_wh[:, 0:128], wh_hw, ident)
            nc.tensor.transpose(ps_wh[:, 128:256], whc_hw, ident)
            whT = whpool.tile([128, 128], F32)  # [w, h]
            whcT = whpool.tile([128, 128], F32)
            nc.scalar.copy(out=whT, in_=ps_wh[:, 0:128])
            nc.scalar.copy(out=whcT, in_=ps_wh[:, 128:256])
            whTr = whT.rearrange("p (ch hh hl) -> p ch hh hl", ch=NCH, hh=8, hl=4)
            whcTr = whcT.rearrange("p (ch hh hl) -> p ch hh hl", ch=NCH, hh=8, hl=4)
            whr = whrep.rearrange(
                "p (ch hl hh m) -> p ch hh hl m", ch=NCH, hl=4, hh=8, m=MID
            )
            whcr = whcrep.rearrange(
                "p (ch hl hh m) -> p ch hh hl m", ch=NCH, hl=4, hh=8, m=MID
            )
            for mi in range(MID):
                nc.vector.tensor_copy(out=whr[:, :, :, :, mi], in_=whTr)
                nc.gpsimd.tensor_copy(out=whcr[:, :, :, :, mi], in_=whcTr)

        X3 = x3pool.tile([128, HP * MID], BF16)  # [w, (hp, m)]
        X3r = X3.rearrange("p (hp m) -> p hp m", m=MID)

        def emit_reduce(ch):
            """x_low for 32 h rows -> X3 rows (via PE stream + XBAR).

            One PSUM bank accumulates all 16 matmuls: variant (t, half) of
            WRP has wrT_half at columns 16t..16t+16 (zeros elsewhere), and
            streams rows h = h0 + 8j + t, so psum = [(t,m), (j4, w)].
            """
            h0 = ch * CH
            xt0, xt1 = xts[ch]
            xr = [
                xt0.rearrange("c (j t) w -> c t j w", t=8),
                xt1.rearrange("c (j t) w -> c t j w", t=8),
            ]
            xl = xlpool.tile([128, 512], BF16, tag="xl")
            if "reduce" in st:
                ps = psrpool.tile([128, 512], F32, tag="psr")
                for t in range(8):
                    for half in range(2):
                        nc.tensor.matmul(
                            ps,
                            lhsT=wrp[:, (t * 2 + half) * 128:(t * 2 + half + 1) * 128],
                            rhs=xr[half][:, t, :, :],
                            start=(t == 0 and half == 0),
                            stop=(t == 7 and half == 1),
                        )
                nc.scalar.copy(out=xl, in_=ps)
            if "xbar1" in st:
                xslab = X3[:, (PAD + h0) * MID:(PAD + h0 + CH) * MID].rearrange(
                    "p (j b) -> p j b", b=128
                )
                eng = nc.sync if ch % 2 == 0 else nc.scalar
                eng.dma_start_transpose(out=xslab, in_=xl)
                if ch == 0:
                    # top reflect: hp 0,1,2 <- hp 6,5,4  (h -k <- h k)
                    for k in range(1, PAD + 1):
                        nc.scalar.copy(
                            out=X3r[:, PAD - k, :], in_=X3r[:, PAD + k, :]
                        )
                if ch == NCH - 1:
                    # bottom reflect: h 127+k <- h 127-k
                    for k in range(1, PAD + 1):
                        nc.scalar.copy(
                            out=X3r[:, PAD + H - 1 + k, :],
                            in_=X3r[:, PAD + H - 1 - k, :],
                        )

        def emit_chunk(ch):
            """conv + blend + pack + expand + store for 32 output rows."""
            h0 = ch * CH

            def xsl(i):
                return X3[:, (h0 + i) * MID:(h0 + i) * MID + 512]

            Ya = ypool.tile([128, 512], BF16, tag="ya")
            Yb = ypool.tile([128, 512], BF16, tag="yb")
            if "hconv" in st:
                # symmetric taps: shared pair sums on gpsimd, FMA on DVE
                s_tiles = []
                for i in range(3):
                    s = ypool.tile([128, 512], BF16, tag=f"s{i}")
                    nc.gpsimd.tensor_add(out=s, in0=xsl(i), in1=xsl(K - 1 - i))
                    s_tiles.append(s)
                nc.vector.tensor_scalar_mul(Ya, xsl(3), float(GH_A[3]))
                nc.vector.tensor_scalar_mul(Yb, xsl(3), float(GH_B[3]))
                for i in range(3):
                    nc.vector.scalar_tensor_tensor(
                        out=Ya, in0=s_tiles[i], scalar=float(GH_A[i]),
                        in1=Ya, op0=mybir.AluOpType.mult,
                        op1=mybir.AluOpType.add,
                    )
                    nc.vector.scalar_tensor_tensor(
                        out=Yb, in0=s_tiles[i], scalar=float(GH_B[i]),
                        in1=Yb, op0=mybir.AluOpType.mult,
                        op1=mybir.AluOpType.add,
                    )
            # W-pass: Za = TwA @ Ya, Zb = TwB @ Yb; rhs streamed in
            # h-shuffled order so psum cols are (hl, hh, m)
            psa = pswpool.tile([128, 512], F32, tag="psw")
            psb = pswpool.tile([128, 512], F32, tag="psw")
            if "wpass" in st:
                ya_shuf = Ya.rearrange("p (hh hl m) -> p hl hh m", hh=8, hl=4, m=MID)
                yb_shuf = Yb.rearrange("p (hh hl m) -> p hl hh m", hh=8, hl=4, m=MID)
                nc.tensor.matmul(psa, lhsT=twtA, rhs=ya_shuf, start=True, stop=True)
                nc.tensor.matmul(psb, lhsT=twtB, rhs=yb_shuf, start=True, stop=True)
            # blend: OL = whrep*Za + whcrep*Zb (each op reads one PSUM bank)
            OL = bpool.tile([128, 512], BF16, tag="ol")
            if "blend" in st:
                t1 = bpool.tile([128, 512], F32, tag="t1")
                nc.vector.tensor_mul(
                    out=t1, in0=psa, in1=whrep[:, h0 * MID:h0 * MID + 512]
                )
                t2 = bpool.tile([128, 512], F32, tag="t2")
                nc.vector.tensor_mul(
                    out=t2, in0=psb, in1=whcrep[:, h0 * MID:h0 * MID + 512]
                )
                nc.vector.tensor_add(out=OL, in0=t1, in1=t2)
            # pack: XBAR transpose -> OLp [(hh,m), (hl, w)]
            olp = olppool.tile([128, 4, 128], BF16, tag="olp")
            if "xbar2" in st:
                nc.sync.dma_start_transpose(out=olp, in_=OL)
            # expand: 16 matmuls; WET variant (hh, cc) selects 16 of 128
            # contraction rows; out rows h0+4hh .. h0+4hh+3 contiguous
            ost0 = opool.tile([128, CH * W], BF16, tag="ost0")
            ost1 = opool.tile([128, CH * W], BF16, tag="ost1")
            ost = [ost0, ost1]
            for hh in range(8):
                for cc in range(2):
                    if "expand" in st:
                        pso = psopool.tile([128, 512], F32, tag="pso")
                        nc.tensor.matmul(
                            pso,
                            lhsT=wet[:, (hh * 2 + cc) * 128:(hh * 2 + cc + 1) * 128],
                            rhs=olp,
                            start=True, stop=True,
                        )
                        idx = hh * 2 + cc
                        dst = ost[cc][:, hh * 512:hh * 512 + 512]
                        if idx % 2 == 0:
                            nc.scalar.copy(out=dst, in_=pso)
                        else:
                            nc.vector.tensor_copy(out=dst, in_=pso)
            if "outdma" in st:
                for cc in range(2):
                    nc.sync.dma_start(
                        out=out_dram[cc * 128:(cc + 1) * 128, h0:h0 + CH, :],
                        in_=ost[cc].rearrange("c (h w) -> c h w", w=W),
                    )

        # interleaved emission: chunk ch depends on reduce groups ch and ch+1
        emit_reduce(0)
        emit_reduce(1)
        emit_chunk(0)
        emit_xdma(2)
        emit_reduce(2)
        emit_chunk(1)
        emit_xdma(3)
        emit_reduce(3)
        emit_chunk(2)
        emit_chunk(3)

    if split_multiwaits:
        _split_multiwaits(nc)
    return nc


def _split_multiwaits(nc):
    """Walrus in this toolchain accepts at most one sync-wait per
    instruction; hoist extras onto same-engine nops just before it."""
    n_new = 0
    for f in nc.m.functions:
        for bb in f.blocks:
            out, changed = [], False
            for ins in bb.instructions:
                si = ins.sync_info
                if si is not None and len(si.on_wait) > 1:
                    changed = True
                    waits = list(si.on_wait)
                    for w in waits[:-1]:
                        n_new += 1
                        nop = bass_rust.InstNoOp(
                            name=f"I-mwsplit-{n_new}", engine=ins.engine
                        )
                        nop.sync_info = mybir.SyncInfo(on_wait=[w], on_update=[])
                        out.append(nop)
                    ins.sync_info = mybir.SyncInfo(
                        on_wait=[waits[-1]], on_update=list(si.on_update)
                    )
                out.append(ins)
            if changed:
                bb.instructions = out
    return n_new


_NC = None


def _get_nc():
    global _NC
    if _NC is None:
        _NC = build_nc()
    return _NC


def make_in_maps(x, angle_map, w_reduce, w_expand):
    wrT = np.ascontiguousarray(w_reduce.T.astype(np.float32))  # [C, MID]
    # WRP variant (t, half): wrT_half at out-columns 16t..16t+16, else 0
    wrp = np.zeros((128, 16 * 128), np.float32)
    for t in range(8):
        for half in range(2):
            wrp[:, (t * 2 + half) * 128 + 16 * t:
                (t * 2 + half) * 128 + 16 * t + MID] = wrT[
                half * 128:(half + 1) * 128
            ]
    # WET[p=(hh,m), (sel*2+cc)*128 + cl] = w_expand[cc*128+cl, m] if hh==sel
    weT = w_expand.T.astype(np.float32)  # [MID, C]
    wet = np.zeros((128, 16 * 128), np.float32)
    for p in range(128):
        hh, m = p // 16, p % 16
        for cc in range(2):
            wet[p, (hh * 2 + cc) * 128:(hh * 2 + cc + 1) * 128] = weT[
                m, cc * 128:(cc + 1) * 128
            ]
    consts = {
        "WRP": np.ascontiguousarray(wrp).astype(NPBF),
        "TwTA": TWTA.astype(NPBF),
        "TwTB": TWTB.astype(NPBF),
        "ident": IDENT,
        "WET": np.ascontiguousarray(wet).astype(NPBF),
    }
    return [
        {
            "x": np.ascontiguousarray(x[i]).astype(NPBF),
            "angle": np.ascontiguousarray(
                (
                    np.mod(
                        2.0 * angle_map[i].astype(np.float64)
                        + math.pi / 2 + math.pi,
                        2 * math.pi,
                    )
                    - math.pi
                ).astype(np.float32)
            ),
            **consts,
        }
        for i in range(B)
    ]


def kernel(x, angle_map, w_reduce, w_expand):
    nc = _get_nc()
    in_maps = make_in_maps(x, angle_map, w_reduce, w_expand)
    res = run_bass_kernel_spmd(nc, in_maps, core_ids=list(range(B)))
    return np.stack([r["out"] for r in res.results]).astype(np.float32)


# revision 17
# speedup vs baseline: 2.0881x; 1.0482x over previous
"""Trainium2 Bass kernel for DynamicDirectionalConv.

Math (per batch b):
  x_low = einsum('chw,mc->mhw', x, w_reduce)                 # 1x1 reduce C=256->16
  w_h   = cos(angle)^2
  out_low = w_h * (x_low (*) BASE_H) + (1-w_h) * (x_low (*) BASE_V)
  out   = einsum('mhw,cm->chw', out_low, w_expand)           # 1x1 expand 16->256

The per-pixel blend factors out of the tap sum, and both base kernels
are axis-aligned separable Gaussians -> rank-1 7x7 convs, reflect pad.

Sharding: data-parallel over batch, 1 batch per NeuronCore (B=8).

Layout pipeline (per core, per 32-row h-chunk):
  x [c, (h,w)] bf16 --PE stream (wrT stationary)--> x_low [m, (h,w)] bf16
  --XBAR dma transpose--> X3 [w, (hp, m)] bf16 (reflect-padded rows)
  H-pass: symmetric-tap FMA chains (gpsimd pair-sums + DVE STT), bf16
  W-pass: banded reflect matrices on PE, rhs streamed in h-shuffled
    column order (hl*128 + hh*16 + m, h_local = 4*hh + hl)
  blend with cos^2(angle) (whrep pre-shuffled to match)
  --XBAR dma transpose--> OLp [(hh,m), (hl, w)] bf16
  expand: 16 matmuls/chunk, zero-padded weight variants select hh;
    out rows h-contiguous in groups of 4 -> big contiguous out DMA
"""

import math

import numpy as np

import concourse.bass as bass
import concourse.tile as tile
from concourse import mybir
import bass_rust
from concourse.bass_utils import run_bass_kernel_spmd

B, C, H, W, MID = 8, 256, 128, 128, 16
K, PAD = 7, 3
F32 = mybir.dt.float32
F32R = mybir.dt.float32r
BF16 = mybir.dt.bfloat16
NPBF = mybir.dt.np(BF16)

HP = H + 2 * PAD  # 134 padded rows
CH = 32           # h rows per chunk
NCH = H // CH     # 4 chunks

ALL_STAGES = frozenset(
    ["wh", "indma", "reduce", "xbar1", "hconv", "wpass", "blend",
     "xbar2", "expand", "outdma"]
)


# ----------------------------------------------------------------- host consts
def _host_consts():
    ax = np.linspace(-(K // 2), K // 2, K, dtype=np.float64)
    e_w = np.exp(-(ax**2) / (2 * 2.5**2))  # wide gaussian (sigma 2.5)
    e_n = np.exp(-(ax**2) / (2 * 1.0**2))  # narrow gaussian (sigma 1.0)
    # BASE_H[i,j] = e_w[i]*e_n[j]/(S+eps); BASE_V[i,j] = e_n[i]*e_w[j]/(S+eps)
    s_h = float((np.outer(e_w, e_n)).sum()) + 1e-8
    s_v = float((np.outer(e_n, e_w)).sum()) + 1e-8
    gh_A = e_w.astype(np.float32)          # h-taps, kernel A
    gh_B = e_n.astype(np.float32)          # h-taps, kernel B
    gw_A = e_n / s_h                       # w-taps (normalized), kernel A
    gw_B = e_w / s_v

    def refl(t):
        if t < 0:
            return -t
        if t > W - 1:
            return 2 * (W - 1) - t
        return t

    def banded(g):
        T = np.zeros((W, W), dtype=np.float64)
        for wo in range(W):
            for j in range(K):
                T[wo, refl(wo + j - PAD)] += g[j]
        return T.astype(np.float32)

    TwA = banded(gw_A)  # out = TwA @ Y  (w-conv with reflect)
    TwB = banded(gw_B)
    ident = np.eye(128, dtype=np.float32)
    return gh_A, gh_B, np.ascontiguousarray(TwA.T), np.ascontiguousarray(TwB.T), ident


GH_A, GH_B, TWTA, TWTB, IDENT = _host_consts()


# ----------------------------------------------------------------- bass module
def build_nc(split_multiwaits=True, loop_n=1, stages=ALL_STAGES):
    st = frozenset(stages)
    nc = bass.Bass()

    x_in = nc.dram_tensor("x", [C, H, W], BF16, kind="ExternalInput")
    ang_in = nc.dram_tensor("angle", [H, W], F32, kind="ExternalInput")
    wrp_in = nc.dram_tensor("WRP", [128, 16 * 128], BF16, kind="ExternalInput")
    twtA_in = nc.dram_tensor("TwTA", [128, 128], BF16, kind="ExternalInput")
    twtB_in = nc.dram_tensor("TwTB", [128, 128], BF16, kind="ExternalInput")
    ident_in = nc.dram_tensor("ident", [128, 128], F32, kind="ExternalInput")
    wet_in = nc.dram_tensor("WET", [128, 16 * 128], BF16, kind="ExternalInput")
    out_dram = nc.dram_tensor("out", [C, H, W], BF16, kind="ExternalOutput")

    from contextlib import ExitStack

    with tile.TileContext(nc) as tc, ExitStack() as es:
        consts = es.enter_context(tc.tile_pool(name="consts", bufs=1))
        xpool = es.enter_context(tc.tile_pool(name="xpool", bufs=3))
        xlpool = es.enter_context(tc.tile_pool(name="xlow", bufs=2))
        x3pool = es.enter_context(tc.tile_pool(name="x3", bufs=1))
        ypool = es.enter_context(tc.tile_pool(name="y", bufs=2))
        bpool = es.enter_context(tc.tile_pool(name="blend", bufs=2))
        olppool = es.enter_context(tc.tile_pool(name="olp", bufs=2))
        opool = es.enter_context(tc.tile_pool(name="ostage", bufs=2))
        whpool = es.enter_context(tc.tile_pool(name="wh", bufs=2))
        psrpool = es.enter_context(tc.tile_pool(name="psr", bufs=2, space="PSUM"))
        pswpool = es.enter_context(tc.tile_pool(name="psw", bufs=2, space="PSUM"))
        psopool = es.enter_context(tc.tile_pool(name="pso", bufs=3, space="PSUM"))

        # ---- constants first (small; first reduce needs WRP), then x tiles;
        # HW spreads same-engine DMAs across its 16 queues.
        wrp = consts.tile([128, 16 * 128], BF16)
        twtA = consts.tile([128, 128], BF16)
        twtB = consts.tile([128, 128], BF16)
        ident = consts.tile([128, 128], F32)
        wet = consts.tile([128, 16 * 128], BF16)
        nc.sync.dma_start(out=wrp, in_=wrp_in[:])
        nc.sync.dma_start(out=twtA, in_=twtA_in[:])
        nc.sync.dma_start(out=twtB, in_=twtB_in[:])
        nc.sync.dma_start(out=ident, in_=ident_in[:])
        nc.sync.dma_start(out=wet, in_=wet_in[:])

        xts = {}

        def emit_xdma(ch):
            h0 = ch * CH
            xt0 = xpool.tile([128, CH, W], BF16, tag="xt0")
            xt1 = xpool.tile([128, CH, W], BF16, tag="xt1")
            if "indma" in st:
                for r in range(2):
                    hr = h0 + r * 16
                    nc.sync.dma_start(out=xt0[:, r * 16:r * 16 + 16, :],
                                      in_=x_in[0:128, hr:hr + 16, :])
                    nc.sync.dma_start(out=xt1[:, r * 16:r * 16 + 16, :],
                                      in_=x_in[128:256, hr:hr + 16, :])
            xts[ch] = (xt0, xt1)

        emit_xdma(0)
        emit_xdma(1)

        loop_cm = tc.For_i(0, loop_n, 1) if loop_n > 1 else None
        if loop_cm is not None:
            es.enter_context(loop_cm)

        # ---- w_h = cos(angle)^2, replicated over m in shuffled col order.
        # host passes angle pre-mapped to wrap(2a + pi/2);
        # cos(a)^2 = 0.5 + 0.5*sin(2a + pi/2)
        # whrep[w, ch*512 + hl*128 + hh*16 + m] = wh[h= 32ch+4hh+hl, w]
        # whcrep = same for (1 - w_h); blend reads each PSUM bank alone.
        whrep = consts.tile([128, H * MID], F32)
        whcrep = consts.tile([128, H * MID], F32)
        if "wh" in st:
            ang = whpool.tile([128, W], F32)  # [h, w]
            nc.sync.dma_start(out=ang, in_=ang_in[:])
            csq = whpool.tile([128, W], F32)
            nc.scalar.activation(
                csq, ang, mybir.ActivationFunctionType.Sin,
                bias=0.0, scale=1.0,
            )
            wh_hw = whpool.tile([128, W], F32)
            nc.scalar.activation(
                wh_hw, csq, mybir.ActivationFunctionType.Copy,
                bias=0.5, scale=0.5,
            )
            whc_hw = whpool.tile([128, W], F32)
            nc.scalar.activation(
                whc_hw, csq, mybir.ActivationFunctionType.Copy,
                bias=0.5, scale=-0.5,
            )
            ps_wh = psopool.tile([128, 512], F32, tag="pso")
            nc.tensor.transpose(ps_wh[:, 0:128], wh_hw, ident)
            nc.tensor.transpose(ps_wh[:, 128:256], whc_hw, ident)
            whT = whpool.tile([128, 128], F32)  # [w, h]
            whcT = whpool.tile([128, 128], F32)
            nc.scalar.copy(out=whT, in_=ps_wh[:, 0:128])
            nc.scalar.copy(out=whcT, in_=ps_wh[:, 128:256])
            whTr = whT.rearrange("p (ch hh hl) -> p ch hh hl", ch=NCH, hh=8, hl=4)
            whcTr = whcT.rearrange("p (ch hh hl) -> p ch hh hl", ch=NCH, hh=8, hl=4)
            whr = whrep.rearrange(
                "p (ch hl hh m) -> p ch hh hl m", ch=NCH, hl=4, hh=8, m=MID
            )
            whcr = whcrep.rearrange(
                "p (ch hl hh m) -> p ch hh hl m", ch=NCH, hl=4, hh=8, m=MID
            )
            for mi in range(MID):
                nc.vector.tensor_copy(out=whr[:, :, :, :, mi], in_=whTr)
                nc.gpsimd.tensor_copy(out=whcr[:, :, :, :, mi], in_=whcTr)

        X3 = x3pool.tile([128, HP * MID], BF16)  # [w, (hp, m)]
        X3r = X3.rearrange("p (hp m) -> p hp m", m=MID)

        def emit_reduce(ch):
            """x_low for 32 h rows -> X3 rows (via PE stream + XBAR).

            One PSUM bank accumulates all 16 matmuls: variant (t, half) of
            WRP has wrT_half at columns 16t..16t+16 (zeros elsewhere), and
            streams rows h = h0 + 8j + t, so psum = [(t,m), (j4, w)].
            """
            h0 = ch * CH
            xt0, xt1 = xts[ch]
            xr = [
                xt0.rearrange("c (j t) w -> c t j w", t=8),
                xt1.rearrange("c (j t) w -> c t j w", t=8),
            ]
            xl = xlpool.tile([128, 512], BF16, tag="xl")
            if "reduce" in st:
                ps = psrpool.tile([128, 512], F32, tag="psr")
                for t in range(8):
                    for half in range(2):
                        nc.tensor.matmul(
                            ps,
                            lhsT=wrp[:, (t * 2 + half) * 128:(t * 2 + half + 1) * 128],
                            rhs=xr[half][:, t, :, :],
                            start=(t == 0 and half == 0),
                            stop=(t == 7 and half == 1),
                        )
                nc.scalar.copy(out=xl, in_=ps)
            if "xbar1" in st:
                xslab = X3[:, (PAD + h0) * MID:(PAD + h0 + CH) * MID].rearrange(
                    "p (j b) -> p j b", b=128
                )
                eng = nc.sync if ch % 2 == 0 else nc.scalar
                eng.dma_start_transpose(out=xslab, in_=xl)
                if ch == 0:
                    # top reflect: hp 0,1,2 <- hp 6,5,4  (h -k <- h k)
                    for k in range(1, PAD + 1):
                        nc.scalar.copy(
                            out=X3r[:, PAD - k, :], in_=X3r[:, PAD + k, :]
                        )
                if ch == NCH - 1:
                    # bottom reflect: h 127+k <- h 127-k
                    for k in range(1, PAD + 1):
                        nc.scalar.copy(
                            out=X3r[:, PAD + H - 1 + k, :],
                            in_=X3r[:, PAD + H - 1 - k, :],
                        )

        ys = {}

        def emit_hconv(ch):
            """symmetric-tap h-conv for 32 rows: pair sums + FMA tree."""
            h0 = ch * CH

            def xsl(i):
                return X3[:, (h0 + i) * MID:(h0 + i) * MID + 512]

            Ya = ypool.tile([128, 512], BF16, tag="ya")
            Yb = ypool.tile([128, 512], BF16, tag="yb")
            ys[ch] = (Ya, Yb)
            if "hconv" not in st:
                return
            s_tiles = []
            for i in range(3):
                s = ypool.tile([128, 512], BF16, tag=f"s{i}")
                nc.gpsimd.tensor_add(out=s, in0=xsl(i), in1=xsl(K - 1 - i))
                s_tiles.append(s)
            for Y, G in ((Ya, GH_A), (Yb, GH_B)):
                c3 = ypool.tile([128, 512], BF16, tag="c3")
                p2 = ypool.tile([128, 512], BF16, tag="p2t")
                nc.vector.tensor_scalar_mul(c3, xsl(3), float(G[3]))
                nc.vector.tensor_scalar_mul(p2, s_tiles[2], float(G[2]))
                a1 = ypool.tile([128, 512], BF16, tag="a1")
                nc.vector.scalar_tensor_tensor(
                    out=a1, in0=s_tiles[0], scalar=float(G[0]),
                    in1=c3, op0=mybir.AluOpType.mult,
                    op1=mybir.AluOpType.add,
                )
                a2 = ypool.tile([128, 512], BF16, tag="a2")
                nc.vector.scalar_tensor_tensor(
                    out=a2, in0=s_tiles[1], scalar=float(G[1]),
                    in1=p2, op0=mybir.AluOpType.mult,
                    op1=mybir.AluOpType.add,
                )
                nc.vector.tensor_add(out=Y, in0=a1, in1=a2)

        def emit_rest(ch):
            """w-pass + blend + pack + expand + store for 32 output rows."""
            h0 = ch * CH
            Ya, Yb = ys[ch]
            # W-pass: Za = TwA @ Ya, Zb = TwB @ Yb; rhs streamed in
            # h-shuffled order so psum cols are (hl, hh, m)
            psa = pswpool.tile([128, 512], F32, tag="psw")
            psb = pswpool.tile([128, 512], F32, tag="psw")
            if "wpass" in st:
                ya_shuf = Ya.rearrange("p (hh hl m) -> p hl hh m", hh=8, hl=4, m=MID)
                yb_shuf = Yb.rearrange("p (hh hl m) -> p hl hh m", hh=8, hl=4, m=MID)
                nc.tensor.matmul(psa, lhsT=twtA, rhs=ya_shuf, start=True, stop=True)
                nc.tensor.matmul(psb, lhsT=twtB, rhs=yb_shuf, start=True, stop=True)
            # blend: OL = whrep*Za + whcrep*Zb (each op reads one PSUM bank)
            OL = bpool.tile([128, 512], BF16, tag="ol")
            if "blend" in st:
                t1 = bpool.tile([128, 512], F32, tag="t1")
                nc.vector.tensor_mul(
                    out=t1, in0=psa, in1=whrep[:, h0 * MID:h0 * MID + 512]
                )
                t2 = bpool.tile([128, 512], F32, tag="t2")
                nc.vector.tensor_mul(
                    out=t2, in0=psb, in1=whcrep[:, h0 * MID:h0 * MID + 512]
                )
                nc.vector.tensor_add(out=OL, in0=t1, in1=t2)
            # pack: XBAR transpose -> OLp [(hh,m), (hl, w)]
            olp = olppool.tile([128, 4, 128], BF16, tag="olp")
            if "xbar2" in st:
                nc.sync.dma_start_transpose(out=olp, in_=OL)
            # expand: 16 matmuls; WET variant (hh, cc) selects 16 of 128
            # contraction rows; out rows h0+4hh .. h0+4hh+3 contiguous
            ost0 = opool.tile([128, CH * W], BF16, tag="ost0")
            ost1 = opool.tile([128, CH * W], BF16, tag="ost1")
            ost = [ost0, ost1]
            for hh in range(8):
                for cc in range(2):
                    if "expand" in st:
                        pso = psopool.tile([128, 512], F32, tag="pso")
                        nc.tensor.matmul(
                            pso,
                            lhsT=wet[:, (hh * 2 + cc) * 128:(hh * 2 + cc + 1) * 128],
                            rhs=olp,
                            start=True, stop=True,
                        )
                        idx = hh * 2 + cc
                        dst = ost[cc][:, hh * 512:hh * 512 + 512]
                        if idx % 2 == 0:
                            nc.scalar.copy(out=dst, in_=pso)
                        else:
                            nc.vector.tensor_copy(out=dst, in_=pso)
            if "outdma" in st:
                for cc in range(2):
                    nc.sync.dma_start(
                        out=out_dram[cc * 128:(cc + 1) * 128, h0:h0 + CH, :],
                        in_=ost[cc].rearrange("c (h w) -> c h w", w=W),
                    )

        # interleaved emission, keeping each engine's in-order queue from
        # blocking ready work: all reduces run ahead of the chunk bodies
        # (chunk ch depends on reduce groups ch and ch+1)
        emit_reduce(0)
        emit_reduce(1)
        emit_hconv(0)
        emit_xdma(2)
        emit_reduce(2)
        emit_rest(0)
        emit_hconv(1)
        emit_xdma(3)
        emit_reduce(3)
        emit_rest(1)
        emit_hconv(2)
        emit_rest(2)
        emit_hconv(3)
        emit_rest(3)

    if split_multiwaits:
        _split_multiwaits(nc)
    return nc


def _split_multiwaits(nc):
    """Walrus in this toolchain accepts at most one sync-wait per
    instruction; hoist extras onto same-engine nops just before it."""
    n_new = 0
    for f in nc.m.functions:
        for bb in f.blocks:
            out, changed = [], False
            for ins in bb.instructions:
                si = ins.sync_info
                if si is not None and len(si.on_wait) > 1:
                    changed = True
                    waits = list(si.on_wait)
                    for w in waits[:-1]:
                        n_new += 1
                        nop = bass_rust.InstNoOp(
                            name=f"I-mwsplit-{n_new}", engine=ins.engine
                        )
                        nop.sync_info = mybir.SyncInfo(on_wait=[w], on_update=[])
                        out.append(nop)
                    ins.sync_info = mybir.SyncInfo(
                        on_wait=[waits[-1]], on_update=list(si.on_update)
                    )
                out.append(ins)
            if changed:
                bb.instructions = out
    return n_new


_NC = None


def _get_nc():
    global _NC
    if _NC is None:
        _NC = build_nc()
    return _NC


def make_in_maps(x, angle_map, w_reduce, w_expand):
    wrT = np.ascontiguousarray(w_reduce.T.astype(np.float32))  # [C, MID]
    # WRP variant (t, half): wrT_half at out-columns 16t..16t+16, else 0
    wrp = np.zeros((128, 16 * 128), np.float32)
    for t in range(8):
        for half in range(2):
            wrp[:, (t * 2 + half) * 128 + 16 * t:
                (t * 2 + half) * 128 + 16 * t + MID] = wrT[
                half * 128:(half + 1) * 128
            ]
    # WET[p=(hh,m), (sel*2+cc)*128 + cl] = w_expand[cc*128+cl, m] if hh==sel
    weT = w_expand.T.astype(np.float32)  # [MID, C]
    wet = np.zeros((128, 16 * 128), np.float32)
    for p in range(128):
        hh, m = p // 16, p % 16
        for cc in range(2):
            wet[p, (hh * 2 + cc) * 128:(hh * 2 + cc + 1) * 128] = weT[
                m, cc * 128:(cc + 1) * 128
            ]
    consts = {
        "WRP": np.ascontiguousarray(wrp).astype(NPBF),
        "TwTA": TWTA.astype(NPBF),
        "TwTB": TWTB.astype(NPBF),
        "ident": IDENT,
        "WET": np.ascontiguousarray(wet).astype(NPBF),
    }
    return [
        {
            "x": np.ascontiguousarray(x[i]).astype(NPBF),
            "angle": np.ascontiguousarray(
                (
                    np.mod(
                        2.0 * angle_map[i].astype(np.float64)
                        + math.pi / 2 + math.pi,
                        2 * math.pi,
                    )
                    - math.pi
                ).astype(np.float32)
            ),
            **consts,
        }
        for i in range(B)
    ]


def kernel(x, angle_map, w_reduce, w_expand):
    nc = _get_nc()
    in_maps = make_in_maps(x, angle_map, w_reduce, w_expand)
    res = run_bass_kernel_spmd(nc, in_maps, core_ids=list(range(B)))
    return np.stack([r["out"] for r in res.results]).astype(np.float32)
